# revision 22
# baseline (speedup 1.0000x reference)
"""AttentionBlock kernel for 8 Trainium2 NeuronCores (Bass/Tile).

Problem (hardcoded shapes): x [16, 512, 32, 32] fp32, GroupNorm(32 groups,
eps=1e-5) -> 1x1-conv QKV (qkv_w [1536,512], qkv_b) -> 8-head attention over
T=1024 positions (head dim 64) -> 1x1-conv proj -> residual add.

Sharding: pure data-parallel over batch; each of the 8 cores handles 2
batches end-to-end; weights replicated; no collectives.

Per-core dataflow (per batch, all layouts channel-on-partition [128, ko, T]):
  1. GroupNorm stats per channel via bn_stats/bn_aggr, group reduction via a
     tiny constant matmul (G: [512,32] one-hot/16), broadcast back to
     channels via a second constant matmul (B = G^T one-hot), then
     tensor_scalar normalize.  norm_w/norm_b are folded into the QKV weights
     host-side, the 1/sqrt(64) attention scale and the q bias are folded into
     Wq/bq, the k bias is dropped (softmax shift invariance), and the v bias
     is folded into the proj bias.
  2. q,k = Wqk @ h as [128, T] head-pairs (head h occupies partitions
     64*(h%2)..); v^T computed directly as h^T @ Wv^T (no transposes needed).
  3. Per head: St = k^T q in [s, t] layout (K=64 row-tiled, pair-packed via
     tile_position), exp on ScalarE (psum->sbuf), AV+denominator in one
     matmul with lhsT = [v^T | ones] (denominator lands replicated on the
     opposite 64 partitions), reciprocal_approx_fast, one sbuf->sbuf DMA to
     lane-shift the reciprocal onto the numerator partitions, one
     tensor_tensor multiply.
  4. proj matmul + (residual + proj bias) add, DMA out.
"""

import numpy as np

B, C, T = 16, 512, 1024
NH, CH = 8, 64
NG = 32
EPS = 1e-5
NCORES = 8
BPC = B // NCORES  # batches per core
KO = C // 128      # channel chunks

# --- dtype configuration -------------------------------------------------
# 'f32'  : plain float32 matmuls (4 cycles/row on PE)
# 'f32r' : float32 data, matmul operands bitcast to float32r (1 cycle/row)
# 'bf16' : operands stored/cast to bfloat16 (1 cycle/row)
MM_QKV = 'f32r'   # h, wqkT, wvT operand treatment (qkv + v^T matmuls)
MM_ATT = 'f32r'   # q, k, expSt, vT operand treatment (St + AV matmuls)
MM_PROJ = 'f32r'  # a, wpT operand treatment (proj matmul)
TRACE = False
ATT_BLOCKED = True  # St for a head-pair blocked before AVs (fewer PE
                    # tiling-mode switches); needs 16 live expSt tiles.


def _npdt(mode):
    import ml_dtypes
    return np.dtype(ml_dtypes.bfloat16) if mode == 'bf16' else np.float32


def _build_nc():
    import concourse.bass as bass
    import concourse.tile as tile
    from concourse import bacc, mybir
    from contextlib import ExitStack

    f32 = mybir.dt.float32
    f32r = mybir.dt.float32r
    bf16 = mybir.dt.bfloat16

    def mmdt(mode):
        # float32r tiles: same bytes as fp32, but the producing op rounds on
        # write and the PE runs the matmul at full (1 cycle/row) rate.
        return {'bf16': bf16, 'f32r': f32r, 'f32': f32}[mode]

    def mm_ap(ap, mode):
        return ap

    dt_h = mmdt(MM_QKV)    # h tile dtype (rhs of qkv, lhsT of v^T)
    dt_att = mmdt(MM_ATT)  # q, k, expSt, vT tiles
    dt_a = mmdt(MM_PROJ)   # a tile

    # Bacc (not raw Bass): its finalize() runs move_matmul_waits_to_ldweights
    # + generate_event_semaphores, which split multi-sem waits into the 1-wait
    # form walrus codegen requires.
    nc = bacc.Bacc()
    AF = mybir.ActivationFunctionType
    ALU = mybir.AluOpType

    x_d = nc.dram_tensor("x", [BPC, 128, KO, T], f32, kind="ExternalInput")
    wqk_d = nc.dram_tensor("wqkT", [128, KO, 2 * C], mmdt(MM_QKV), kind="ExternalInput")
    wv_d = nc.dram_tensor("wvT", [128, KO, C], mmdt(MM_QKV), kind="ExternalInput")
    wp_d = nc.dram_tensor("wpT", [128, KO, C], mmdt(MM_PROJ), kind="ExternalInput")
    bq_d = nc.dram_tensor("bq", [128, KO], f32, kind="ExternalInput")
    bp_d = nc.dram_tensor("bp", [128, KO], f32, kind="ExternalInput")
    g_d = nc.dram_tensor("gmat", [128, KO, NG], f32, kind="ExternalInput")
    b_d = nc.dram_tensor("bmat", [NG, KO, 128], f32, kind="ExternalInput")
    ones_d = nc.dram_tensor("ones", [128, 64], mmdt(MM_ATT), kind="ExternalInput")
    out_d = nc.dram_tensor("out", [BPC, 128, KO, T], f32, kind="ExternalOutput")

    with tile.TileContext(nc) as tc, ExitStack() as ctx:
        consts = ctx.enter_context(tc.tile_pool(name="consts", bufs=1))
        xp = ctx.enter_context(tc.tile_pool(name="xp", bufs=1))
        hp = ctx.enter_context(tc.tile_pool(name="hp", bufs=1))
        qkp = ctx.enter_context(tc.tile_pool(name="qkp", bufs=1))
        vtp = ctx.enter_context(tc.tile_pool(name="vtp", bufs=1))
        esp = ctx.enter_context(tc.tile_pool(name="esp", bufs=(16 if ATT_BLOCKED else 4)))
        rp = ctx.enter_context(tc.tile_pool(name="rp", bufs=1))
        ap_ = ctx.enter_context(tc.tile_pool(name="ap", bufs=1))
        gnp = ctx.enter_context(tc.tile_pool(name="gnp", bufs=2))
        psQ = ctx.enter_context(tc.tile_pool(name="psQ", bufs=2, space="PSUM"))
        psS = ctx.enter_context(tc.tile_pool(name="psS", bufs=2, space="PSUM"))
        psB = ctx.enter_context(tc.tile_pool(name="psB", bufs=1, space="PSUM"))

        # constants
        wqk_sb = consts.tile([128, KO, 2 * C], mmdt(MM_QKV))
        nc.sync.dma_start(wqk_sb[:], wqk_d[:])
        wv_sb = consts.tile([128, KO, C], mmdt(MM_QKV))
        nc.sync.dma_start(wv_sb[:], wv_d[:])
        wp_sb = consts.tile([128, KO, C], mmdt(MM_PROJ))
        nc.sync.dma_start(wp_sb[:], wp_d[:])
        bq_sb = consts.tile([128, KO], f32)
        nc.sync.dma_start(bq_sb[:], bq_d[:])
        bp_sb = consts.tile([128, KO], f32)
        nc.sync.dma_start(bp_sb[:], bp_d[:])
        g_sb = consts.tile([128, KO, NG], f32)
        nc.sync.dma_start(g_sb[:], g_d[:])
        bm_sb = consts.tile([NG, KO, 128], f32)
        nc.sync.dma_start(bm_sb[:], b_d[:])

        # v^T lhsT buffer: per head-pair p the 192 columns are
        # [vT_even(64) | ones(64) | vT_odd(64)]; head 2p uses cols 0:128 of
        # the block ([vT|ones]) and head 2p+1 uses cols 64:192 ([ones|vT]).
        vt_sb = vtp.tile([128, 8, 4, 192], dt_att)
        # ones blocks loaded via DMA (broadcast AP from a small DRAM constant)
        # rather than memset: keeps the write on vt_sb's own tensor handle so
        # Tile orders it against the AV matmuls.
        ones_src = bass.AP(tensor=ones_d, offset=0,
                           ap=[[64, 128], [0, 32], [1, 64]])
        vt_flat = vt_sb[:].rearrange("p a b w -> p (a b) w")
        nc.sync.dma_start(vt_flat[:, :, 64:128], ones_src)

        eps_sb = consts.tile([NG, 1], f32)
        nc.vector.memset(eps_sb[:], EPS)

        for b in range(BPC):
            x_sb = xp.tile([128, KO, T], f32, tag="x")
            nc.sync.dma_start(x_sb[:], x_d[b])

            # ---------------- GroupNorm ----------------
            rhs3 = gnp.tile([128, KO, 3], f32, tag="rhs3")
            for ko in range(KO):
                stats = gnp.tile([128, 2, 6], f32, tag="stats")
                for j in range(2):
                    nc.vector.bn_stats(out=stats[:, j, :], in_=x_sb[:, ko, 512 * j:512 * (j + 1)])
                nc.vector.bn_aggr(out=rhs3[:, ko, 0:2], in_=stats[:])
                nc.vector.tensor_mul(rhs3[:, ko, 2:3], rhs3[:, ko, 0:1], rhs3[:, ko, 0:1])
            gps = psQ.tile([NG, 3], f32, tag="mm")
            for ko in range(KO):
                nc.tensor.matmul(gps[:], g_sb[:, ko, :], rhs3[:, ko, :],
                                 start=(ko == 0), stop=(ko == KO - 1))
            # var = E[var] + E[mean^2] - mean^2 ; rstd = 1/sqrt(var+eps)
            gq = gnp.tile([NG, 3], f32, tag="gq")
            nc.vector.tensor_copy(gq[:], gps[:])
            gtmp = gnp.tile([NG, 2], f32, tag="gtmp")
            gst2 = gnp.tile([NG, 2], f32, tag="gst2")
            nc.vector.tensor_copy(gst2[:, 0:1], gq[:, 0:1])
            nc.vector.tensor_add(gtmp[:, 0:1], gq[:, 1:2], gq[:, 2:3])
            nc.vector.tensor_mul(gtmp[:, 1:2], gq[:, 0:1], gq[:, 0:1])
            nc.vector.tensor_sub(gtmp[:, 0:1], gtmp[:, 0:1], gtmp[:, 1:2])
            nc.scalar.activation(gtmp[:, 1:2], gtmp[:, 0:1], AF.Sqrt, bias=eps_sb[:])
            nc.vector.reciprocal(gst2[:, 1:2], gtmp[:, 1:2])
            bst_ps = psQ.tile([128, 2 * KO], f32, tag="mm")
            for ko in range(KO):
                nc.tensor.matmul(bst_ps[:, 2 * ko:2 * ko + 2], bm_sb[:, ko, :], gst2[:],
                                 start=True, stop=True)
            bst = gnp.tile([128, 2 * KO], f32, tag="bst_sb")
            nc.vector.tensor_copy(bst[:], bst_ps[:])
            h_sb = hp.tile([128, KO, T], dt_h, tag="h")
            for ko in range(KO):
                nc.vector.tensor_scalar(
                    out=h_sb[:, ko, :], in0=x_sb[:, ko, :],
                    scalar1=bst[:, 2 * ko:2 * ko + 1], scalar2=bst[:, 2 * ko + 1:2 * ko + 2],
                    op0=ALU.subtract, op1=ALU.mult)
            # pre-add proj bias to residual x (x := x + bp per channel)
            for ko in range(KO):
                nc.vector.tensor_scalar(
                    out=x_sb[:, ko, :], in0=x_sb[:, ko, :],
                    scalar1=bp_sb[:, ko:ko + 1], scalar2=None, op0=ALU.add)

            # ---------------- QKV (q,k) ----------------
            q_sb = qkp.tile([128, KO, T], dt_att, tag="q")
            k_sb = qkp.tile([128, KO, T], dt_att, tag="k")
            for m in range(8):
                for half in range(2):
                    pq = psQ.tile([128, 512], f32, tag="mm")
                    for ko in range(KO):
                        nc.tensor.matmul(
                            pq[:], mm_ap(wqk_sb[:, ko, 128 * m:128 * (m + 1)], MM_QKV),
                            mm_ap(h_sb[:, ko, 512 * half:512 * (half + 1)], MM_QKV),
                            start=(ko == 0), stop=(ko == KO - 1))
                    if m < 4:
                        nc.vector.tensor_scalar(
                            out=q_sb[:, m, 512 * half:512 * (half + 1)], in0=pq[:],
                            scalar1=bq_sb[:, m:m + 1], scalar2=None, op0=ALU.add)
                    else:
                        nc.vector.tensor_copy(k_sb[:, m - 4, 512 * half:512 * (half + 1)], pq[:])

            # ---------------- v^T ----------------
            for tc_i in range(8):
                pv = psQ.tile([128, 512], f32, tag="mm")
                for ko in range(KO):
                    nc.tensor.matmul(
                        pv[:], mm_ap(h_sb[:, ko, 128 * tc_i:128 * (tc_i + 1)], MM_QKV),
                        mm_ap(wv_sb[:, ko, :], MM_QKV),
                        start=(ko == 0), stop=(ko == KO - 1))
                pvv = pv[:].rearrange("p (h c) -> p h c", c=CH)
                nc.vector.tensor_copy(vt_sb[:, tc_i, :, 0:64], pvv[:, 0:NH:2, :])
                nc.vector.tensor_copy(vt_sb[:, tc_i, :, 128:192], pvv[:, 1:NH:2, :])

            # ---------------- attention ----------------
            def emit_st(h):
                """St = k^T q for head h -> list of 8 expSt sbuf tiles."""
                p, e = h // 2, h % 2
                b0 = 64 * e          # this head's partition base (N rows)
                es_tiles = []
                for sc in range(8):
                    st = psS.tile([128, T], f32, tag="st")
                    for half in range(2):
                        nc.tensor.matmul(
                            st[:, 512 * half:512 * (half + 1)],
                            mm_ap(k_sb[b0:b0 + 64, p, 128 * sc:128 * (sc + 1)], MM_ATT),
                            mm_ap(q_sb[b0:b0 + 64, p, 512 * half:512 * (half + 1)], MM_ATT),
                            start=True, stop=True, tile_position=(b0, 0))
                    es = esp.tile([128, T], dt_att, tag="es")
                    nc.scalar.activation(es[:], st[:], AF.Exp)
                    es_tiles.append(es)
                return es_tiles

            def emit_av(h, es_tiles, a_sb):
                p, e = h // 2, h % 2
                b0, b1 = 64 * e, 64 * (1 - e)
                av = [psB.tile([128, 512], f32, tag=f"av{i}", name=f"av{i}") for i in range(2)]
                for sc in range(8):
                    es = es_tiles[sc]
                    for half in range(2):
                        nc.tensor.matmul(
                            av[half][:], mm_ap(vt_sb[:, sc, p, 64 * e:64 * e + 128], MM_ATT),
                            mm_ap(es[:, 512 * half:512 * (half + 1)], MM_ATT),
                            start=(sc == 0), stop=(sc == 7))
                r = rp.tile([128, T], f32, tag="r")
                for half in range(2):
                    nc.vector.reciprocal_approx_fast(
                        out=r[b1:b1 + 64, 512 * half:512 * (half + 1)],
                        in_=av[half][b1:b1 + 64, :])
                nc.sync.dma_start(out=r[b0:b0 + 64, :], in_=r[b1:b1 + 64, :])
                for half in range(2):
                    nc.vector.tensor_tensor(
                        out=a_sb[b0:b0 + 64, p, 512 * half:512 * (half + 1)],
                        in0=av[half][b0:b0 + 64, :],
                        in1=r[b0:b0 + 64, 512 * half:512 * (half + 1)], op=ALU.mult)

            a_sb = ap_.tile([128, KO, T], dt_a, tag="a")
            if ATT_BLOCKED:
                for p in range(4):
                    es0 = emit_st(2 * p)
                    es1 = emit_st(2 * p + 1)
                    emit_av(2 * p, es0, a_sb)
                    emit_av(2 * p + 1, es1, a_sb)
            else:
                for h in range(NH):
                    emit_av(h, emit_st(h), a_sb)

            # ---------------- proj + residual ----------------
            o_sb = hp.tile([128, KO, T], f32, tag="h")
            for m in range(KO):
                for half in range(2):
                    po = psQ.tile([128, 512], f32, tag="mm")
                    for ko in range(KO):
                        nc.tensor.matmul(
                            po[:], mm_ap(wp_sb[:, ko, 128 * m:128 * (m + 1)], MM_PROJ),
                            mm_ap(a_sb[:, ko, 512 * half:512 * (half + 1)], MM_PROJ),
                            start=(ko == 0), stop=(ko == KO - 1))
                    nc.vector.tensor_add(
                        o_sb[:, m, 512 * half:512 * (half + 1)], po[:],
                        x_sb[:, m, 512 * half:512 * (half + 1)])
            nc.sync.dma_start(out_d[b], o_sb[:])

    if not nc.is_finalized():
        nc.finalize()
    return nc


def _prep_inputs(x, norm_w, norm_b, qkv_w, qkv_b, proj_w, proj_b):
    """Fold norms/biases/scale into weights; reshape for the kernel layout."""
    f = np.float32
    x = np.asarray(x, f)
    nw = np.asarray(norm_w, f)
    nb = np.asarray(norm_b, f)
    qkv_w = np.asarray(qkv_w, f)
    qkv_b = np.asarray(qkv_b, f)
    proj_w = np.asarray(proj_w, f)
    proj_b = np.asarray(proj_b, f)

    Wq, Wk, Wv = qkv_w[0:C], qkv_w[C:2 * C], qkv_w[2 * C:3 * C]
    bqv, bkv, bvv = qkv_b[0:C], qkv_b[C:2 * C], qkv_b[2 * C:3 * C]
    scale = f(1.0 / np.sqrt(CH))
    Wq_e = (Wq * nw[None, :]) * scale
    bq_e = (Wq @ nb + bqv) * scale
    Wk_e = Wk * nw[None, :]          # k bias dropped (softmax shift invariance)
    Wv_e = Wv * nw[None, :]
    bv_e = Wv @ nb + bvv
    bp_e = proj_b + proj_w @ bv_e    # v bias folded into proj bias

    def chan_chunks(vec):  # [C] -> [128, KO]
        return np.ascontiguousarray(vec.reshape(KO, 128).T)

    def lhsT_chunks(wT, dtype):  # [C, M] -> [128, KO, M]
        return np.ascontiguousarray(
            wT.reshape(KO, 128, wT.shape[1]).transpose(1, 0, 2)).astype(dtype)

    wqkT = np.concatenate([Wq_e, Wk_e], axis=0).T  # [C, 1024]
    gm = np.zeros((C, NG), f)
    gm[np.arange(C), np.arange(C) // (C // NG)] = 1.0 / (C // NG)
    bm = np.zeros((NG, C), f)
    bm[np.arange(C) // (C // NG), np.arange(C)] = 1.0

    dqkv = _npdt(MM_QKV)
    dproj = _npdt(MM_PROJ)
    shared = {
        "wqkT": lhsT_chunks(wqkT, dqkv),
        "wvT": lhsT_chunks(Wv_e.T, dqkv),
        "wpT": lhsT_chunks(proj_w.T, dproj),
        "bq": chan_chunks(bq_e),
        "bp": chan_chunks(bp_e),
        "gmat": np.ascontiguousarray(
            gm.reshape(KO, 128, NG).transpose(1, 0, 2)),
        "bmat": np.ascontiguousarray(bm.reshape(NG, KO, 128)),
        "ones": np.ones((128, 64), _npdt(MM_ATT)),
    }
    xr = x.reshape(B, C, T)
    in_maps = []
    for c in range(NCORES):
        xc = xr[c * BPC:(c + 1) * BPC].reshape(BPC, KO, 128, T).transpose(0, 2, 1, 3)
        m = dict(shared)
        m["x"] = np.ascontiguousarray(xc)
        in_maps.append(m)
    return in_maps


def kernel(x, norm_w, norm_b, qkv_w, qkv_b, proj_w, proj_b):
    from concourse.bass_utils import run_bass_kernel_spmd

    in_maps = _prep_inputs(x, norm_w, norm_b, qkv_w, qkv_b, proj_w, proj_b)
    nc = _build_nc()
    res = run_bass_kernel_spmd(nc, in_maps, core_ids=list(range(NCORES)), trace=TRACE)
    kernel.last_results = res
    outs = []
    for c in range(NCORES):
        oc = res.results[c]["out"]  # [BPC, 128, KO, T]
        outs.append(np.asarray(oc).transpose(0, 2, 1, 3).reshape(BPC, C, T))
    full = np.concatenate(outs, axis=0).reshape(B, C, 32, 32).astype(np.float32)
    return full
